# revision 9
# baseline (speedup 1.0000x reference)
"""Bass/Trainium2 kernel for NF4-dequant (QLoRA-style) SwiGLU MLP.

Computation (matches the bitsandbytes-NF4 reference):
    dq_i = nf4_quant_dequant(w_i)   (per-64-block absmax scaling)
    out  = dq3-proj( silu(x @ dq1^T) * (x @ dq2^T) )

Sharding: tensor-parallel over the ffn dim H=11008 across 8 cores.
H is split in 64-aligned shards of width [1408 x4, 1344 x4]; the 1344
shards are zero-padded to 1408 so every core runs the same program.
Each core computes a full [T, D] partial of the down-projection; the
host sums the 8 partials (the TP all-reduce).
"""

import os
import sys

import numpy as np

if not os.path.isdir(os.path.join(os.path.dirname(os.path.abspath(__file__)), "concourse")):
    for _p in ("/opt/trn_rl_repo",):
        if os.path.isdir(_p) and _p not in sys.path:
            sys.path.insert(0, _p)

import concourse.bass as bass
import concourse.mybir as mybir
import concourse.tile as tile
from concourse import bacc
from concourse.bass_utils import run_bass_kernel_spmd
from concourse.masks import make_identity

F32 = mybir.dt.float32
BF16 = mybir.dt.bfloat16
OP = mybir.AluOpType

# NF4 codebook (bitsandbytes), exactly as in the reference.
NF4_CODE = np.array(
    [
        -1.0, -0.6961928009986877, -0.5250730514526367, -0.39491748809814453,
        -0.28444138169288635, -0.18477343022823334, -0.09105003625154495, 0.0,
        0.07958029955625534, 0.16093020141124725, 0.24611230194568634,
        0.33791524171829224, 0.44070982933044434, 0.5626170039176941,
        0.7229568362236023, 1.0,
    ],
    dtype=np.float32,
)
# Bucket boundaries and level deltas, computed in f32 like the reference.
NF4_BOUNDS = ((NF4_CODE[:-1] + NF4_CODE[1:]) * np.float32(0.5)).astype(np.float32)
NF4_DELTAS = (NF4_CODE[1:] - NF4_CODE[:-1]).astype(np.float32)

BLK = 64  # NF4 blocksize

# Problem dims (hardcoded per the harness contract).
D = 4096
T_FULL = 4096  # 2 * 2048 tokens
H_FULL = 11008
N_CORES = 8
HP = 1408  # padded per-core shard of H (22 blocks of 64)
# 64-aligned shard widths summing to 11008.
SHARD_W = [1408, 1408, 1408, 1408, 1344, 1344, 1344, 1344]
SHARD_START = [0, 1408, 2816, 4224, 5632, 6976, 8320, 9664]

T_BLK = 1024  # token block held in SBUF per pass
NSUB = 512  # matmul moving free dim / PSUM bank width


def _emit_dequant_tile(
    nc,
    pools,
    w_ap,  # DRAM AP of the natural-layout weight shard [rows, free]
    row0,  # first row of this 128-row tile
    col0,  # first col of this chunk
    cw,  # chunk width (multiple of 64)
    scratch,  # DRAM scratch tensor AP for the transposed dequant output
    scratch_write,  # fn(j_block_abs, sbuf_tile_ap) -> (dram_ap) for transposed block
    identity,
):
    """Dequantize a [128, cw] natural tile and write its transpose (bf16)."""
    pw, pa, pv, pacc, ptmp, pdq, pps, pqt = pools
    nblk = cw // BLK

    wt = pw.tile([128, cw], F32, tag="wt")
    nc.sync.dma_start(wt[:], w_ap[row0 : row0 + 128, col0 : col0 + cw])
    w3 = wt[:].rearrange("p (b i) -> p b i", i=BLK)

    amax = pa.tile([128, nblk], F32, tag="amax")
    nc.vector.tensor_reduce(
        amax[:], w3, axis=mybir.AxisListType.X, op=OP.max, apply_absolute_value=True
    )
    aclamp = pa.tile([128, nblk], F32, tag="aclamp")
    nc.vector.tensor_scalar_max(aclamp[:], amax[:], 1e-35)
    recip = pa.tile([128, nblk], F32, tag="recip")
    nc.vector.reciprocal(recip[:], aclamp[:])

    r_b = recip[:].unsqueeze(2).broadcast_to([128, nblk, BLK])
    a_b = amax[:].unsqueeze(2).broadcast_to([128, nblk, BLK])

    vn = pv.tile([128, cw], F32, tag="vn")
    vn3 = vn[:].rearrange("p (b i) -> p b i", i=BLK)
    nc.vector.tensor_tensor(vn3, w3, r_b, OP.mult)

    acc = pacc.tile([128, cw], F32, tag="acc")
    nc.vector.tensor_scalar(
        acc[:], vn[:], float(NF4_BOUNDS[0]), float(NF4_DELTAS[0]), OP.is_gt, OP.mult
    )
    tmp = ptmp.tile([128, cw], F32, tag="tmp")
    for j in range(1, 15):
        nc.vector.tensor_scalar(
            tmp[:], vn[:], float(NF4_BOUNDS[j]), float(NF4_DELTAS[j]), OP.is_gt, OP.mult
        )
        nc.vector.tensor_tensor(acc[:], acc[:], tmp[:], OP.add)

    dq = pdq.tile([128, cw], BF16, tag="dq")
    dq3 = dq[:].rearrange("p (b i) -> p b i", i=BLK)
    acc3 = acc[:].rearrange("p (b i) -> p b i", i=BLK)
    # dq = (acc + code[0]) * absmax
    nc.vector.scalar_tensor_tensor(dq3, acc3, -1.0, a_b, OP.add, OP.mult)

    # transpose 128x128 blocks via PE and store
    for jb in range(cw // 128):
        ps = pps.tile([128, 128], BF16, tag="ps")
        nc.tensor.transpose(ps[:], dq[:, jb * 128 : (jb + 1) * 128], identity[:])
        qt = pqt.tile([128, 128], BF16, tag="qt")
        nc.scalar.copy(qt[:], ps[:])
        nc.sync.dma_start(scratch_write((col0 + jb * 128) // 128), qt[:])


def _build_program():
    """Build the full 8-core SPMD program (bf16 matmul variant)."""
    nc = bacc.Bacc("TRN2", target_bir_lowering=False, debug=False, num_devices=N_CORES)

    xT = nc.dram_tensor("xT", [D, T_FULL], F32, kind="ExternalInput").ap()
    w1s = nc.dram_tensor("w1s", [HP, D], F32, kind="ExternalInput").ap()
    w2s = nc.dram_tensor("w2s", [HP, D], F32, kind="ExternalInput").ap()
    w3s = nc.dram_tensor("w3s", [D, HP], F32, kind="ExternalInput").ap()
    out = nc.dram_tensor("out", [T_FULL, D], F32, kind="ExternalOutput").ap()

    KT = D // 128  # 32 k-tiles over D
    HT = HP // 128  # 11 h-tiles
    NTB = T_FULL // T_BLK  # token blocks

    from contextlib import ExitStack

    with tile.TileContext(nc) as tc, ExitStack() as ctx:
        dram = ctx.enter_context(tc.tile_pool(name="dram", bufs=1, space="DRAM"))
        # transposed dequant scratch; layouts chosen for big contiguous
        # per-partition DMA lines on the read side.
        # s12[w][h_tile]: [128 (d_in), KT*128 (k-tile major, h_in minor)]
        s1 = dram.tile([HT, 128, KT, 128], BF16)
        s2 = dram.tile([HT, 128, KT, 128], BF16)
        # s3[h_tile]: [128 (h_in), D]
        s3 = dram.tile([HT, 128, D], BF16)

        const = ctx.enter_context(tc.tile_pool(name="const", bufs=1))
        identity = const.tile([128, 128], BF16)
        make_identity(nc, identity[:])

        pw = ctx.enter_context(tc.tile_pool(name="pw", bufs=2))
        pa = ctx.enter_context(tc.tile_pool(name="pa", bufs=2))
        pv = ctx.enter_context(tc.tile_pool(name="pv", bufs=2))
        pacc = ctx.enter_context(tc.tile_pool(name="pacc", bufs=2))
        ptmp = ctx.enter_context(tc.tile_pool(name="ptmp", bufs=2))
        pdq = ctx.enter_context(tc.tile_pool(name="pdq", bufs=2))
        pps = ctx.enter_context(tc.tile_pool(name="pps", bufs=8, space="PSUM"))
        pqt = ctx.enter_context(tc.tile_pool(name="pqt", bufs=4))
        dq_pools = (pw, pa, pv, pacc, ptmp, pdq, pps, pqt)

        px = ctx.enter_context(tc.tile_pool(name="px", bufs=2))
        pxb = ctx.enter_context(tc.tile_pool(name="pxb", bufs=KT))
        pl = ctx.enter_context(tc.tile_pool(name="pl", bufs=2))
        ph = ctx.enter_context(tc.tile_pool(name="ph", bufs=HT + 1))
        psl = ctx.enter_context(tc.tile_pool(name="psl", bufs=2))
        pr3 = ctx.enter_context(tc.tile_pool(name="pr3", bufs=3))
        pob = ctx.enter_context(tc.tile_pool(name="pob", bufs=4))

        def dequant_weight(w_ap, rows, chunks, scratch_fn):
            for i in range(rows // 128):
                for (ch, cw) in chunks:
                    _emit_dequant_tile(
                        nc, dq_pools, w_ap, i * 128, ch, cw,
                        None, lambda jb, i=i: scratch_fn(i, jb), identity,
                    )

        # phase-1 writers for the three scratch layouts
        def s12_write(s, h_tile, k_tile):
            return s[h_tile, :, k_tile, :]

        def s3_write(d_tile, h_tile):
            return s3[h_tile, :, d_tile * 128 : (d_tile + 1) * 128]

        def phase2(tb):
            # load + cast x block
            xk = []
            for k in range(KT):
                xf = px.tile([128, T_BLK], F32, tag="xf")
                nc.sync.dma_start(
                    xf[:], xT[k * 128 : (k + 1) * 128, tb * T_BLK : (tb + 1) * T_BLK]
                )
                xb = pxb.tile([128, T_BLK], BF16, tag="xb")
                nc.vector.tensor_copy(xb[:], xf[:])
                xk.append(xb)
            hsb = []
            for h in range(HT):
                l1 = pl.tile([128, KT * 128], BF16, tag="l1")
                nc.sync.dma_start(l1[:], s1[h, :, :, :].rearrange("p k i -> p (k i)"))
                l2 = pl.tile([128, KT * 128], BF16, tag="l2")
                nc.sync.dma_start(l2[:], s2[h, :, :, :].rearrange("p k i -> p (k i)"))
                ht = ph.tile([128, T_BLK], BF16, tag="ht")
                for c in range(T_BLK // NSUB):
                    pg = pps.tile([128, NSUB], F32, tag="ps")
                    pu = pps.tile([128, NSUB], F32, tag="ps")
                    for k in range(KT):
                        nc.tensor.matmul(
                            pg[:],
                            l1[:, k * 128 : (k + 1) * 128],
                            xk[k][:, c * NSUB : (c + 1) * NSUB],
                            start=(k == 0),
                            stop=(k == KT - 1),
                        )
                    for k in range(KT):
                        nc.tensor.matmul(
                            pu[:],
                            l2[:, k * 128 : (k + 1) * 128],
                            xk[k][:, c * NSUB : (c + 1) * NSUB],
                            start=(k == 0),
                            stop=(k == KT - 1),
                        )
                    sl = psl.tile([128, NSUB], BF16, tag="sl")
                    nc.scalar.activation(
                        sl[:], pg[:], mybir.ActivationFunctionType.Silu
                    )
                    nc.vector.tensor_tensor(
                        ht[:, c * NSUB : (c + 1) * NSUB], sl[:], pu[:], OP.mult
                    )
                hsb.append(ht)
            return hsb

        def phase3(tb, hsb):
            for dc in range(D // NSUB):
                po = [
                    pps.tile([128, NSUB], F32, tag="ps", name=f"po{tt}")
                    for tt in range(T_BLK // 128)
                ]
                for k in range(HT):
                    r3 = pr3.tile([128, NSUB], BF16, tag="r3")
                    nc.sync.dma_start(r3[:], s3[k, :, dc * NSUB : (dc + 1) * NSUB])
                    for tt in range(T_BLK // 128):
                        nc.tensor.matmul(
                            po[tt][:],
                            hsb[k][:, tt * 128 : (tt + 1) * 128],
                            r3[:],
                            start=(k == 0),
                            stop=(k == HT - 1),
                        )
                for tt in range(T_BLK // 128):
                    ob = pob.tile([128, NSUB], F32, tag="ob")
                    nc.scalar.copy(ob[:], po[tt][:])
                    nc.sync.dma_start(
                        out[
                            tb * T_BLK + tt * 128 : tb * T_BLK + (tt + 1) * 128,
                            dc * NSUB : (dc + 1) * NSUB,
                        ],
                        ob[:],
                    )

        # Emission order: w1, w2, first token block's gate/up (PE work) while
        # w3 dequant (DVE) runs, then the rest.
        w12_chunks = [(0, 1024), (1024, 1024), (2048, 1024), (3072, 1024)]
        w3_chunks = [(0, 640), (640, 768)]
        dequant_weight(w1s, HP, w12_chunks, lambda i, jb: s12_write(s1, i, jb))
        dequant_weight(w2s, HP, w12_chunks, lambda i, jb: s12_write(s2, i, jb))
        hsb0 = phase2(0)
        dequant_weight(w3s, D, w3_chunks, s3_write)
        phase3(0, hsb0)
        for tb in range(1, NTB):
            hsb = phase2(tb)
            phase3(tb, hsb)

    nc.compile()
    return nc


_CACHED_NC = None
LAST_RESULTS = None


def _shard_inputs(x, w1, w2, w3):
    xT = np.ascontiguousarray(x.reshape(T_FULL, D).T, dtype=np.float32)
    in_maps = []
    for c in range(N_CORES):
        s, w = SHARD_START[c], SHARD_W[c]
        w1c = np.zeros((HP, D), dtype=np.float32)
        w1c[:w] = w1[s : s + w]
        w2c = np.zeros((HP, D), dtype=np.float32)
        w2c[:w] = w2[s : s + w]
        w3c = np.zeros((D, HP), dtype=np.float32)
        w3c[:, :w] = w3[:, s : s + w]
        in_maps.append({"xT": xT, "w1s": w1c, "w2s": w2c, "w3s": w3c})
    return in_maps


def kernel(x, w1, w2, w3):
    global _CACHED_NC, LAST_RESULTS
    assert x.shape == (2, 2048, D) and w1.shape == (H_FULL, D)
    if _CACHED_NC is None:
        _CACHED_NC = _build_program()
    in_maps = _shard_inputs(x, w1, w2, w3)
    res = run_bass_kernel_spmd(
        _CACHED_NC,
        in_maps,
        core_ids=list(range(N_CORES)),
        trace=os.environ.get("KERNEL_TRACE", "") == "1",
    )
    LAST_RESULTS = res
    acc = res.results[0]["out"].astype(np.float32).copy()
    for c in range(1, N_CORES):
        acc += res.results[c]["out"]
    return acc.reshape(2, 2048, D).astype(np.float32)
